# revision 39
# baseline (speedup 1.0000x reference)
"""GCN layer (CustomGraphConv) on 8 trn2 NeuronCores via Bass/Tile.

Math: out = D^{-1/2} (A + I) D^{-1/2} @ X @ W + bias
  - A: [N, N] 0/1 symmetric adjacency (f32 input), N = 8192
  - X: [N, 256] f32, W: [256, 256] f32, bias: [256] f32

Sharding: 1D node partition. Core c owns nodes R_c = [c*C, (c+1)*C), C = N/8,
and receives the column strip A_aug[:, R_c] (self-loops added on the host,
cast to fp8e4 -- exact for 0/1 -- and pre-tiled partition-major).

v4 restructure (vs the AllGather baseline):
  out_own = d_own * ((A_strip^T (ds * X)) @ W) + bias
  i.e. W is applied AFTER neighborhood aggregation, so the X@W GEMM
  (13.6us of PE) leaves the critical path; only a [C,256]@[256,256]
  GEMM (1.7us) remains at the tail.

  Degrees are exchanged with XOR-relative remote_dma broadcasts (~1us)
  instead of a collective AllGather (15us constant overhead on this stack):
  each core issues 8 single-destination broadcast descriptors (dest slot d
  -> physical core = mine XOR d) during the A-load window and fires them
  with one trigger_dma as soon as its own-column degree sums are drained.
  Static SPMD addressing works because the host permutes each core's A/X
  k-tile order into "XOR slot order": SBUF slot d*J+j on core r holds the
  absolute k-tile owner*J+j, owner = L(R(r) XOR d) (R/L = logical<->physical
  NC maps, identity when unavailable; any XOR-form map cancels out).

Device program (identical SPMD program on all 8 cores):
  1. Load A strip as packed fp8 k-tile groups on the sync queue (A first:
     it gates degrees -> everything), then w/bias, then X row-block pairs.
  2. deg_own[p, j] = colsum of the strip via 256 free=1 fp8 DoubleRow
     matmuls (A pair stationary x ones moving), directly p-major in one
     PSUM bank; drained to SBUF.
  3. trigger_dma fires the 8 pre-generated remote_dma broadcasts; every
     core's [128, J] degree shard lands in the right slot of every other
     core's deg_stage. Consumers gate on the remote-arrival semaphore.
  4. ds = S*rsqrt(deg_stage) (own-slot copy computed separately from
     deg_own so own k-tiles never wait on the exchange).
  5. X tiles scaled+split to fp8 hi/lo (hi = fp8(ds*x) on ACT, lo =
     fp8(ds*x - hi) on DVE), feeding
  6. M^T[g, j] += dsXhi_pair^T A_pair + dsXlo_pair^T A_pair: fp8 DoubleRow,
     8 PSUM banks (2 g-halves x 4 j-quarters), one accumulation group each,
     kp-outer so pairs are consumed as conversions produce them. Dummy
     ones x ones matmuls bridge the PE p-state gap after the colsum.
  7. M^T drains to fp16, tiny MW GEMM per j-tile (f16, stationary M^T
     slice, moving W half), epilogue out = d_own' * psum + bias split
     across DVE / ACT+Pool, stores deferred on sync/scalar queues.

Toolchain constraints (inherited from the baseline):
  - walrus caps sync waits at 1/instruction -> _split_dma_waits hoists
    extras onto standalone EventSemaphore instructions.
  - SBUF access patterns keep the partition dim explicit (t[0:1, :]).
  - A matmul accumulation group owns its PSUM bank until `stop`; PSUM
    pools are scoped per phase.
  - fp8 DoubleRow contracts two 128-row subtiles per pass at 0.5
    cycles/row; both operands fp8e4, moving free total <= 512.
"""

import numpy as np
import ml_dtypes

import concourse.bass as bass
import concourse.mybir as mybir
import concourse.tile as tile
from concourse import library_config
from concourse.bass_utils import run_bass_kernel_spmd
from concourse.library_overlay import lower_extended_insts

NCORES = 8
F = 256
S = 64.0  # fp8 block scale for the ds*X hi/lo split (max |S ds x| ~ 70 << 448)
N_DUMMY = 36  # PE p-state bridge between colsum end and first MM' pair

f32 = mybir.dt.float32
fp16 = mybir.dt.float16
fp8 = mybir.dt.float8e4
DR = mybir.MatmulPerfMode.DoubleRow


def _phys_nc_map():
    """logical nc -> physical nc for device 0; identity when the driver is
    unreachable (any XOR-form remap cancels in the slot algebra anyway)."""
    try:
        from concourse.libnrt import get_trn2_nc_mapping

        m = get_trn2_nc_mapping()
        return [m[(0, i)] for i in range(NCORES)]
    except Exception:
        return list(range(NCORES))


def _slot_owner():
    """owner[r][d] = logical core whose degree shard lands in slot d of
    (logical) receiver r: sender s satisfies R(s) XOR d == R(r)."""
    R = _phys_nc_map()
    L = [0] * NCORES
    for i, p in enumerate(R):
        L[p] = i
    return [[L[R[r] ^ d] for d in range(NCORES)] for r in range(NCORES)]


def _build_probe():
    """Tiny SPMD program that reveals the true slot->sender permutation of
    the XOR-relative broadcast fabric: every core broadcasts its (host-fed)
    core index with the same 8 single-dest sends the main kernel uses;
    probe_out[r][0, d] = index of the core whose data lands in slot d."""
    nc = bass.Bass()
    pid = nc.dram_tensor("pid", [128, 1], f32, kind="ExternalInput")
    pout = nc.dram_tensor("probe_out", [128, NCORES], f32, kind="ExternalOutput")
    rsem = nc.alloc_semaphore("probe_recv", num=244)
    lsem = nc.alloc_semaphore("probe_sent", num=245)
    with tile.TileContext(nc) as tc:
        with tc.tile_pool(name="p", bufs=1) as p:
            stage = p.tile([128, NCORES], f32, name="stage")
            own = p.tile([128, 1], f32, name="own")
            ld = nc.sync.dma_start(out=own[:], in_=pid[:])
            nc.gpsimd.load_library(library_config.proxy)
            clr_r = nc.gpsimd.sem_clear(rsem)
            clr_l = nc.gpsimd.sem_clear(lsem)
            for d in range(NCORES):
                rd = [None] * NCORES
                rd[d] = (0, d)
                nc.gpsimd.remote_dma_broadcast(
                    out_ap=stage[:, d : d + 1],
                    in_ap=own[:],
                    remote_sem=rsem,
                    local_sem=lsem,
                    rdests=rd,
                )
            trig = nc.gpsimd.trigger_dma(count=None)
            bass._add_dep_helper(trig.ins, ld.ins, reason="pid before trigger")
            bass._add_dep_helper(trig.ins, clr_r.ins, reason="clear before fire")
            bass._add_dep_helper(trig.ins, clr_l.ins, reason="clear before fire")
            st = nc.sync.dma_start(out=pout[:], in_=stage[:])
    lower_extended_insts(nc)
    _insert_sem_gate(nc, st.ins.name, rsem, 2 * NCORES)
    _split_dma_waits(nc)
    return nc


_OWNER_CACHE = None


def _probe_owner():
    """Runtime slot->owner table: probe the actual device fabric once per
    process; fall back to the libnrt-derived (or identity) XOR map if the
    probe fails validation."""
    global _OWNER_CACHE
    if _OWNER_CACHE is not None:
        return _OWNER_CACHE
    try:
        nc = _build_probe()
        in_maps = [
            {"pid": np.full((128, 1), c, np.float32)} for c in range(NCORES)
        ]
        res = run_bass_kernel_spmd(nc, in_maps, list(range(NCORES)))
        owner = []
        for r in range(NCORES):
            row = [int(v) for v in np.asarray(res.results[r]["probe_out"])[0]]
            assert sorted(row) == list(range(NCORES)), (r, row)
            assert row[0] == r, (r, row)  # slot 0 must be the self-send
            owner.append(row)
    except Exception as e:  # noqa: BLE001 -- any failure means "use static map"
        import logging

        logging.warning(f"slot probe failed ({e!r}); using static XOR map")
        owner = _slot_owner()
    _OWNER_CACHE = owner
    return owner


def _split_dma_waits(nc):
    """Hoist semaphore waits onto standalone EventSemaphore instructions on
    the issuing engine's queue, for any instruction carrying more than one
    (walrus caps sync waits at 1/instruction, 2 for EventSemaphore)."""
    ctr = 0
    for fn in nc.m.functions:
        for bb in fn.blocks:
            new_insts = []
            for inst in bb.instructions:
                si = inst.sync_info
                if (
                    not isinstance(inst, mybir.InstEventSemaphore)
                    and si is not None
                    and len(si.on_wait) > 1
                ):
                    for w in si.on_wait[:-1]:
                        ev = mybir.InstEventSemaphore(
                            name=f"hoistw-{ctr}",
                            engine=inst.engine,
                            ins=[],
                            outs=[],
                            sync_info=mybir.SyncInfo(on_wait=[w], on_update=[]),
                        )
                        ctr += 1
                        new_insts.append(ev)
                    inst.sync_info = mybir.SyncInfo(
                        on_wait=[si.on_wait[-1]], on_update=si.on_update
                    )
                new_insts.append(inst)
            bb.instructions = new_insts


def _insert_sem_gate(nc, before_name: str, sem, value: int):
    """Insert a standalone EventSemaphore wait (sem >= value) directly before
    the named instruction, on the same engine's queue. Applied after Tile
    scheduling: the gate depends on REMOTE cores' RDMA arrivals, which the
    single-core scheduling pass cannot satisfy."""
    for fn in nc.m.functions:
        for bb in fn.blocks:
            for i, inst in enumerate(bb.instructions):
                if inst.name == before_name:
                    ev = mybir.InstEventSemaphore(
                        name=f"rdma-gate-{before_name}",
                        engine=inst.engine,
                        ins=[],
                        outs=[],
                        sync_info=mybir.SyncInfo(
                            on_wait=[
                                mybir.SyncWait(
                                    sync_type="semaphore",
                                    id=sem.num,
                                    ant_name=sem.name,
                                    wait_mode="sem-ge-imm",
                                    wait_value=value,
                                )
                            ],
                            on_update=[],
                        ),
                    )
                    bb.instructions = (
                        bb.instructions[:i] + [ev] + bb.instructions[i:]
                    )
                    return
    raise AssertionError(f"instruction {before_name} not found for sem gate")


def build(n_nodes: int, split_waits: bool = True):
    """Build the SPMD Bass program for one core (all cores identical)."""
    N = n_nodes
    C = N // NCORES  # own nodes per core
    KT = N // 128  # 128-row k tiles
    NP = KT // 2  # k-tile pairs (DoubleRow contracts 2 per pass)
    J = KT // NCORES  # k-tiles per slot == deg columns == out 128-blocks
    JQW = min(C, 256)  # MM' moving j-slice width (DR moving free <= 2*256)
    JQ = C // JQW  # j quarter-slices
    GH = F // 128  # feature halves (g on partitions)
    if KT == 64:
        packs = [32, 16, 8, 4, 2, 2]
    else:
        packs = [KT]
    assert sum(packs) == KT and all(p % 2 == 0 for p in packs)

    nc = bass.Bass()
    # partition-major pre-tiled strip in XOR-slot k order:
    #   a_strip[p, b*C + c] = A_aug[abs_ktile(b)*128 + p, own c]
    a_strip = nc.dram_tensor("a_strip", [128, KT * C], fp8, kind="ExternalInput")
    # X row-blocks in the same XOR-slot k order, fp16
    xn = nc.dram_tensor("xn", [N, F], fp16, kind="ExternalInput")
    w = nc.dram_tensor("w", [F, F], fp16, kind="ExternalInput")
    bias_bc = nc.dram_tensor("bias_bc", [128, F], f32, kind="ExternalInput")
    out = nc.dram_tensor("out", [C, F], f32, kind="ExternalOutput")

    rsem = nc.alloc_semaphore("deg_recv", num=240)
    lsem = nc.alloc_semaphore("deg_sent", num=241)

    with tile.TileContext(nc) as tc:
        with (
            tc.tile_pool(name="persist", bufs=1) as persist,
            tc.tile_pool(name="work", bufs=2) as work,
        ):
            # ---- A strip loads: contiguous packs, first on the sync queue
            # (A gates degrees -> the exchange -> everything).
            a_pk = []
            k2pack = []  # k-tile -> (pack idx, offset within pack)
            k0 = 0
            for g, pk in enumerate(packs):
                t = persist.tile([128, pk * C], fp8, name=f"a{g}")
                nc.sync.dma_start(out=t[:], in_=a_strip[:, k0 * C : (k0 + pk) * C])
                a_pk.append(t)
                for i in range(pk):
                    k2pack.append((g, i))
                k0 += pk

            def a_pair(kp, c0, c1):
                """[128, 2, c1-c0] fp8 AP of k-tile pair kp, cols [c0, c1)."""
                g, i = k2pack[2 * kp]
                v = a_pk[g][:, i * C : (i + 2) * C].rearrange(
                    "p (t c) -> p t c", t=2
                )
                return v[:, :, c0:c1]

            # X pair loads directly after A (the first pairs gate the first
            # conversions; w/bias are only needed at the tail GEMM)
            xv_dram = xn.rearrange("(k p) f -> p k f", p=128)
            x_sb = []
            for kp in range(NP):
                t = persist.tile([128, 2 * F], fp16, name=f"x{kp}")
                nc.sync.dma_start(
                    out=t.rearrange("p (t f) -> p t f", t=2),
                    in_=xv_dram[:, 2 * kp : 2 * kp + 2, :],
                )
                x_sb.append(t)

            w_sb = [persist.tile([128, F], fp16, name=f"w{i}") for i in range(GH)]
            for i in range(GH):
                nc.sync.dma_start(out=w_sb[i][:], in_=w[i * 128 : (i + 1) * 128, :])
            bias_sb = persist.tile([128, F], f32, name="bias")
            nc.sync.dma_start(out=bias_sb[:], in_=bias_bc[:])

            ones = persist.tile([128, 256], fp8, name="ones")
            nc.vector.memset(ones[:], 1.0)
            ones_v = ones.rearrange("p (t m) -> p t m", t=2)

            # ACT table warmer (the sqrt set also holds Copy, so one table
            # covers every ACT op in this kernel)
            act_warm = persist.tile([1, 1], f32, name="act_warm")
            nc.scalar.activation(
                act_warm[:], ones[0:1, 0:1], mybir.ActivationFunctionType.Sqrt
            )

            # Q7 library with remote-DMA desc-gen AND tensor_tensor (used by
            # the epilogue), loaded once up front -- no mid-kernel reload.
            nc.gpsimd.load_library(library_config.proxy)
            # clear the exchange semaphores: kills residue from previous
            # NEFFs/executions. Safe vs in-flight arrivals -- no core can
            # send before its own ~25us A-load + colsum completes.
            clr_r = nc.gpsimd.sem_clear(rsem)
            clr_l = nc.gpsimd.sem_clear(lsem)

            # ---- degree-exchange descriptors: 8 single-dest XOR-relative
            # broadcasts, generated early on the Pool SWDGE ring (desc-gen
            # ~1us each, hidden in the A window); data read at trigger time.
            deg_stage = persist.tile([128, KT], f32, name="deg_stage")
            deg_own = persist.tile([128, J], f32, name="deg_own")
            for d in range(NCORES):
                rd = [None] * NCORES
                rd[d] = (0, d)  # slot d: bit2(slot) == bit2(dtpb) holds
                nc.gpsimd.remote_dma_broadcast(
                    out_ap=deg_stage[:, d * J : (d + 1) * J],
                    in_ap=deg_own[:],
                    remote_sem=rsem,
                    local_sem=lsem,
                    rdests=rd,
                )

            # ---- deg_own[p, j] = colsum of the strip, p-major directly:
            # free=1 DR matmuls (A pair stationary, ones moving), one PSUM
            # bank, one accumulation group. Own-node scales are fused
            # single-op ACT rsqrts straight from PSUM (no DVE hop).
            last_deg_mm = None
            d_own = persist.tile([128, J], f32, name="d_own")
            ds_own = persist.tile([128, J], f32, name="ds_own")
            with tc.tile_pool(name="degpsum", bufs=1, space="PSUM") as degpsum:
                deg_ps = degpsum.tile([128, J], f32, name="deg_ps")
                for kp in range(NP):
                    for jt in range(J):
                        last_deg_mm = nc.tensor.matmul(
                            deg_ps[:, jt : jt + 1],
                            a_pair(kp, jt * 128, (jt + 1) * 128),
                            ones_v[:, :, 0:1],
                            start=(kp == 0 and jt == 0),
                            stop=(kp == NP - 1 and jt == J - 1),
                            perf_mode=DR,
                        )
                deg_copy = nc.vector.tensor_copy(deg_own[:], deg_ps[:])
                # own scale straight from PSUM, in parallel with deg_copy:
                # ds_own = S/sqrt(deg) = sqrt(S^2/deg). d_own is computed
                # later -- it is epilogue-only and its ACT op would sit on
                # the ds_own -> first-hi critical path here.
                r8 = persist.tile([128, J], f32, name="r8")
                nc.vector.reciprocal(r8[:], deg_ps[:])
                nc.scalar.activation(
                    ds_own[:], r8[:], mybir.ActivationFunctionType.Sqrt, scale=S * S
                )

            # ---- fire the exchange (reads deg_own at trigger time; the
            # prep-time in_ap read predates the deg_copy write in emission
            # order, so pin the data dependency on the trigger explicitly)
            trig = nc.gpsimd.trigger_dma(count=None)
            bass._add_dep_helper(trig.ins, deg_copy.ins, reason="data before trigger")
            bass._add_dep_helper(trig.ins, clr_r.ins, reason="clear before fire")
            bass._add_dep_helper(trig.ins, clr_l.ins, reason="clear before fire")

            # ---- conversions + M^T accumulation
            zhi = [persist.tile([128, 2 * F], fp8, name=f"zh{kp}") for kp in range(NP)]
            zlo = [persist.tile([128, 2 * F], fp8, name=f"zl{kp}") for kp in range(NP)]
            ds = persist.tile([128, KT], f32, name="ds")

            def convert_pair(kp):
                """hi/lo fp8 split of ds*X for k-tiles 2kp, 2kp+1."""
                for half in range(2):
                    m = 2 * kp + half
                    dsm = ds_own[:, m : m + 1] if m < J else ds[:, m : m + 1]
                    xm = x_sb[kp][:, half * F : (half + 1) * F]
                    hi = zhi[kp][:, half * F : (half + 1) * F]
                    lo = zlo[kp][:, half * F : (half + 1) * F]
                    nc.scalar.activation(
                        hi, xm, mybir.ActivationFunctionType.Copy, scale=dsm
                    )
                    nc.vector.scalar_tensor_tensor(
                        lo,
                        xm,
                        dsm,
                        hi,
                        mybir.AluOpType.mult,
                        mybir.AluOpType.subtract,
                    )

            # own-slot pairs (need only ds_own): the FIRST TWO are emitted
            # before the gated ds chain so ACT/DVE convert them while the
            # exchange is in flight; the rest come after the gate so the
            # recip/sqrt (ready ~arrival time) don't queue behind 8 ops.
            own_pairs = min(J // 2, NP)
            early = min(2, own_pairs)
            for kp in range(early):
                convert_pair(kp)

            # full ds -- gated on all 8 shards having landed (2 sem incs per
            # single-dest broadcast x 8 senders incl. self). The runtime gate
            # (wait deg_recv >= 16) is inserted post-scheduling by
            # _insert_sem_gate: a wait emitted here would deadlock Tile's
            # single-core scheduling pass, where only the self-send fires.
            ds_op = nc.vector.reciprocal(ds[:], deg_stage[:])
            bass._add_dep_helper(ds_op.ins, trig.ins, reason="ds after trigger")
            nc.scalar.activation(
                ds[:], ds[:], mybir.ActivationFunctionType.Sqrt, scale=S * S
            )
            for kp in range(early, own_pairs):
                convert_pair(kp)

            with tc.tile_pool(name="mtpsum", bufs=1, space="PSUM") as mtpsum:
                mt_ps = [
                    [
                        mtpsum.tile([128, JQW], f32, name=f"mt{gh}_{jq}")
                        for jq in range(JQ)
                    ]
                    for gh in range(GH)
                ]
                # PE p-state bridge: keep the PE busy from colsum end until
                # the first converted pair is ready (queue order holds them
                # between the deg matmuls and the first MM' pass).
                for i in range(N_DUMMY):
                    dmm = nc.tensor.matmul(
                        mt_ps[0][0][:, 0:128],
                        ones_v[:, :, 0:128],
                        ones_v[:, :, 0:128],
                        start=True,
                        stop=True,
                        perf_mode=DR,
                    )
                    if i == 0:
                        bass._add_dep_helper(
                            dmm.ins, last_deg_mm.ins, reason="warm after colsum"
                        )
                for kp in range(NP):
                    if kp >= own_pairs:
                        convert_pair(kp)
                    zh = zhi[kp].rearrange("p (t f) -> p t f", t=2)
                    zl = zlo[kp].rearrange("p (t f) -> p t f", t=2)
                    if kp < NP - 1:
                        for zv in (zh, zl):
                            for gh in range(GH):
                                for jq in range(JQ):
                                    nc.tensor.matmul(
                                        mt_ps[gh][jq][:],
                                        zv[:, :, gh * 128 : (gh + 1) * 128],
                                        a_pair(kp, jq * JQW, (jq + 1) * JQW),
                                        start=(kp == 0 and zv is zh),
                                        stop=False,
                                        perf_mode=DR,
                                    )
                    else:
                        # final pair jq-major with hi/lo interleaved per
                        # bank, so each (gh0,jq)/(gh1,jq) bank pair stops
                        # early and its drain + MW GEMM pipeline into the
                        # remaining accumulation
                        for jq in range(JQ):
                            for gh in range(GH):
                                for zv in (zh, zl):
                                    nc.tensor.matmul(
                                        mt_ps[gh][jq][:],
                                        zv[:, :, gh * 128 : (gh + 1) * 128],
                                        a_pair(kp, jq * JQW, (jq + 1) * JQW),
                                        start=(kp == 0 and zv is zh),
                                        stop=(zv is zl),
                                        perf_mode=DR,
                                    )
                # epilogue-only own scale d_own' = sqrt(1/(S^2 deg)): emitted
                # here so its ACT op never delays the conversion stream
                nc.scalar.activation(
                    d_own[:],
                    r8[:],
                    mybir.ActivationFunctionType.Sqrt,
                    scale=1.0 / (S * S),
                )
                # drains: M^T psum -> fp16 SBUF, jq-major so the MW GEMMs
                # start per quarter; ACT and DVE in parallel per jq (walrus:
                # GPSIMD cannot read PSUM).
                m_t = [
                    persist.tile([128, C], fp16, name=f"mt{gh}") for gh in range(GH)
                ]
                JT = C // 128
                for jq in range(JQ):
                    for gh in range(GH):
                        eng = nc.vector.tensor_copy if gh == 0 else nc.scalar.copy
                        eng(
                            m_t[gh][:, jq * JQW : (jq + 1) * JQW], mt_ps[gh][jq][:]
                        )

            # ---- MW GEMM + epilogue + stores
            with tc.tile_pool(name="outpsum", bufs=1, space="PSUM") as outpsum:
                out_ps = [
                    outpsum.tile([128, F], f32, name=f"ops{jt}") for jt in range(JT)
                ]
                deferred_stores = []
                for jt in range(JT):
                    for gh in range(GH):
                        nc.tensor.matmul(
                            out_ps[jt][:],
                            m_t[gh][:, jt * 128 : (jt + 1) * 128],
                            w_sb[gh][:],
                            start=(gh == 0),
                            stop=(gh == GH - 1),
                        )
                    ot = work.tile([128, F], f32, tag="ot", bufs=8)
                    if jt % 2 == 1:
                        nc.vector.scalar_tensor_tensor(
                            ot[:],
                            out_ps[jt][:],
                            d_own[:, jt : jt + 1],
                            bias_sb[:],
                            mybir.AluOpType.mult,
                            mybir.AluOpType.add,
                        )
                    else:
                        sc = work.tile([128, F], f32, tag="sc", bufs=4)
                        nc.scalar.activation(
                            sc[:],
                            out_ps[jt][:],
                            mybir.ActivationFunctionType.Copy,
                            scale=d_own[:, jt : jt + 1],
                        )
                        nc.gpsimd.tensor_tensor(
                            ot[:], sc[:], bias_sb[:], mybir.AluOpType.add
                        )
                    # all stores on the sync queue: it is idle by now, while
                    # the ACT queue is still draining epilogue scale ops
                    deferred_stores.append((nc.sync, jt, ot))
                for eng, jt, ot in deferred_stores:
                    eng.dma_start(out=out[jt * 128 : (jt + 1) * 128, :], in_=ot[:])
    # raw Bass skips Bacc's codegen_inst_isa_subclasses pass; without it the
    # NEFF compiler sees empty .instr for the extended-inst ISA subclasses
    # (remote_dma desc-gen) -> "ISA wrong length"
    lower_extended_insts(nc)
    _insert_sem_gate(nc, ds_op.ins.name, rsem, 2 * NCORES)
    if split_waits:
        _split_dma_waits(nc)
    return nc


_CACHE = {}


def _get_program(n_nodes: int):
    if n_nodes not in _CACHE:
        _CACHE[n_nodes] = build(n_nodes)
    return _CACHE[n_nodes]


def _prep_inputs(A, inputs, weight, bias, owner=None):
    """Host-side marshaling: shard + XOR-slot k-tile permutation + layout +
    dtype casts."""
    N = A.shape[0]
    C = N // NCORES
    KT = N // 128
    J = KT // NCORES
    A_aug = np.asarray(A, dtype=np.float32)
    idx = np.arange(N)
    A_aug = A_aug.astype(ml_dtypes.float8_e4m3)
    A_aug[idx, idx] = np.float32(1.0)  # reference adds I; A diag is 0
    x16 = np.asarray(inputs, dtype=np.float32).astype(np.float16)
    w16 = np.asarray(weight, dtype=np.float32).astype(np.float16)
    bias_bc = np.ascontiguousarray(
        np.broadcast_to(np.asarray(bias, dtype=np.float32), (128, F))
    )
    if owner is None:
        owner = _slot_owner()
    in_maps = []
    for r in range(NCORES):
        korder = [owner[r][b // J] * J + (b % J) for b in range(KT)]
        strip = A_aug[:, r * C : (r + 1) * C]  # [N, C]
        tiled = np.ascontiguousarray(
            strip.reshape(KT, 128, C)[korder]
            .transpose(1, 0, 2)
            .reshape(128, KT * C)
        )
        xr = np.ascontiguousarray(x16.reshape(KT, 128, F)[korder].reshape(N, F))
        in_maps.append({"a_strip": tiled, "xn": xr, "w": w16, "bias_bc": bias_bc})
    return in_maps


def kernel(A, inputs, weight, bias):
    N = A.shape[0]
    owner = _probe_owner()
    nc = _get_program(N)
    in_maps = _prep_inputs(A, inputs, weight, bias, owner=owner)
    res = run_bass_kernel_spmd(nc, in_maps, list(range(NCORES)))
    return np.concatenate([r["out"] for r in res.results], axis=0)


if __name__ == "__main__":
    # mini self-check with a host reference
    N = 1024
    rng = np.random.default_rng(0)
    A = (rng.random((N, N)) < 0.01).astype(np.float32)
    A = np.maximum(A, A.T)
    np.fill_diagonal(A, 0.0)
    X = rng.standard_normal((N, F)).astype(np.float32)
    W = (rng.random((F, F)).astype(np.float32) / 100.0) - 0.005
    b = (rng.random(F).astype(np.float32) / 100.0) - 0.005

    A_ = A + np.eye(N, dtype=np.float32)
    deg = A_.sum(axis=1)
    d = deg**-0.5
    expected = (d[:, None] * A_ * d[None, :]) @ X @ W + b

    nc = _get_program(N)
    in_maps = _prep_inputs(A, X, W, b)
    res = run_bass_kernel_spmd(nc, in_maps, list(range(NCORES)))
    got = np.concatenate([r["out"] for r in res.results], axis=0)
    err = np.abs(got - expected)
    scale = np.abs(expected).max()
    print("rel err:", err.max() / scale, "nan:", np.isnan(got).sum(), "/", got.size)
